# revision 34
# baseline (speedup 1.0000x reference)
"""Trainium2 Bass kernel for nn_DiffNet (gnn_message_passing).

The reference's per-element "edge MLP" over the meta stack (vi, W, vj)
collapses algebraically: with g = conv1_w.T @ conv2_w[0],
hb = conv1_b@conv2_w[0]+conv2_b[0], z = vi @ W.T (no bias),
s1[b] = sum_i vi[b,i], s2[b] = sum_i vi[b,i]^2:

    out = relu(z+b)*(1 + k2*s1) + k1*z + (k0*s2 + kb*s1)

so the network is 3 matmuls + elementwise.  Distribution: fc1/fc2
replicated (zero-communication), fc3 sharded over its output dim
(32 cols/core); host concatenates the 8 [32,32] shards.

v5 design (v1 fp32 ~33us, v2/v4 fp16 ~25.6us):
 - all PE dataflow fp16 (rel err ~1.5e-3 vs the 2e-2 gate).
 - W-stationary matmuls: z lands [out_feature, batch]; no transposes.
 - DMA: 8 contiguous slices spread across THREE issue queues (gpsimd
   SWDGE for the small lead, sync + scalar HWDGE alternating for the
   weight slices) — a single queue is issue-rate-limited (~0.65us per
   DMA_DIRECT2D issue vs ~0.7us transfer per 256KB slice).
 - broadcast-stats: ones[128,128]-stationary chains accumulate s1/s2
   broadcast across partitions in PSUM; alpha/beta are single-PSUM-input
   DVE ops.
 - per-layer tail: both relus per half on ACT (one vj tile, one writer
   engine), t1=k1*z+beta on DVE, combine h0 on DVE / h1 on GpSimd.
 - every engine queue's op order is pinned with explicit scheduler
   edges: Tile otherwise reorders and can park a DMA-gated op at the
   head of the DVE queue, head-of-line-blocking the whole tail.
 - warmup matmuls from ~t0 (tones memset on GpSimd) lift the PE HAM
   clock gate (1.2 -> 2.4 GHz) before the real chains start.
"""

import sys

if "/opt/trn_rl_repo" not in sys.path:
    sys.path.insert(0, "/opt/trn_rl_repo")

import numpy as np


def _install_ntff_hook_shim():
    """This image's antenv lacks ``axon_hooks``; bass_utils hard-imports it
    when tracing under axon.  Provide the module and register the ctypes
    NTFF hook from trn_agent_boot so ``trace=True`` yields exec_time_ns."""
    import types

    if "antenv.axon_hooks" in sys.modules:
        return
    try:
        import antenv

        mod = types.ModuleType("antenv.axon_hooks")
        _h = [None]
        mod.set_axon_ntff_profile_hook = lambda hook: _h.__setitem__(0, hook)
        mod.get_axon_ntff_profile_hook = lambda: _h[0]
        sys.modules["antenv.axon_hooks"] = mod
        antenv.axon_hooks = mod
        from trn_agent_boot.trn_boot import _ntff_profile_via_ctypes

        mod.set_axon_ntff_profile_hook(
            _ntff_profile_via_ctypes("/opt/axon/libaxon_pjrt.so")
        )
    except Exception:
        pass


_install_ntff_hook_shim()

N_CORES = 8
B = 32
I1, O1, O2, O3 = 1024, 512, 512, 256
O3L = O3 // N_CORES  # fc3 output cols per core
RATE = 0.1
N_WARMUP_MM = 25
MISC_F16 = 24  # misc f32 [128,12] carried as 24 f16 cols of the lead DMA
XW = 8 * B     # x.T cols

_CACHE = {}
LAST_RESULTS = None  # BassKernelResults of the most recent run (for test.py)


def _build(k0, k1, k2, kb):
    import concourse.bacc as bacc
    import concourse.mybir as mybir
    import concourse.tile as tile
    import concourse.bass as bass
    from concourse.tile_rust import add_dep_helper

    f32 = mybir.dt.float32
    f16 = mybir.dt.float16
    AF = mybir.ActivationFunctionType
    ALU = mybir.AluOpType

    nc = bacc.Bacc(
        "TRN2", target_bir_lowering=False, debug=False, num_devices=N_CORES
    )

    lead = nc.declare_dram_parameter(
        "lead", [128, MISC_F16 + XW], f16, isOutput=False
    )
    w1s = [
        nc.declare_dram_parameter(f"w1s{m}", [128, 2048], f16, isOutput=False)
        for m in range(2)
    ]
    w2a = nc.declare_dram_parameter("w2a", [128, 1024], f16, isOutput=False)
    w2b = nc.declare_dram_parameter("w2b", [128, 1024], f16, isOutput=False)
    w3 = nc.declare_dram_parameter("w3m", [128, 4 * O3L], f16, isOutput=False)
    out_d = nc.declare_dram_parameter("out", [O3L, B], f32, isOutput=True)

    # per-engine op-order pinning (Tile's scheduler otherwise reorders)
    _last = {}

    def pin(inst, q):
        if inst is None:
            return inst
        prev = _last.get(q)
        if prev is not None:
            add_dep_helper(inst.ins, prev.ins, sync=False, reason="qorder")
        _last[q] = inst
        return inst

    with tile.TileContext(nc) as tc:
        with (
            tc.tile_pool(name="wts", bufs=1) as wp,
            tc.tile_pool(name="act", bufs=1) as ap,
            tc.tile_pool(name="ps", bufs=1, space=bass.MemorySpace.PSUM) as pp,
        ):
            tlead = wp.tile([128, MISC_F16 + XW], f16, tag="lead")
            tw1 = [
                wp.tile([128, 2048], f16, tag=f"w1{m}", name=f"tw1{m}")
                for m in range(2)
            ]
            tw2a = wp.tile([128, 1024], f16, tag="w2a")
            tw2b = wp.tile([128, 1024], f16, tag="w2b")
            tw3 = wp.tile([128, 4 * O3L], f16, tag="w3")
            tones = wp.tile([128, 128], f16, tag="ones")
            nc.gpsimd.memset(tones[:], 1.0)

            tmisc = tlead[:, 0:MISC_F16].bitcast(f32)      # [128, 12] f32
            tx = tlead[:, MISC_F16 : MISC_F16 + XW]        # [128, 256] f16

            # DMA issues split over the two HWDGE queues in strict global
            # need-order: both w1 halves stream before any w2 bytes so the
            # layer-1 critical path never shares the wire with layer 2.
            nc.sync.dma_start(tlead[:], lead[:])
            nc.scalar.dma_start(tw1[0][:], w1s[0][:])
            nc.sync.dma_start(tw1[1][:], w1s[1][:])
            nc.scalar.dma_start(tw2a[:], w2a[:])
            nc.sync.dma_start(tw2b[:], w2b[:])
            nc.scalar.dma_start(tw3[:], w3[:])

            # PE warmup against the HAM clock gate.
            junk = pp.tile([128, 64], f32, tag="za")
            for _ in range(N_WARMUP_MM):
                pin(nc.tensor.matmul(
                    junk[:], tones[:], tones[:, 0:64], start=True, stop=True
                ), "pe")

            def schain(s_ps, col, src, n_c):
                for c in range(n_c):
                    pin(nc.tensor.matmul(
                        s_ps[:, col : col + B],
                        tones[:],
                        src[:, c * B : (c + 1) * B],
                        start=(c == 0),
                        stop=(c == n_c - 1),
                    ), "pe")

            def zchain(z_ps, col, w_view, woff, a_sb, n_c):
                for ic in range(n_c):
                    pin(nc.tensor.matmul(
                        z_ps[:, col : col + B],
                        w_view[:, woff + ic * 128 : woff + (ic + 1) * 128],
                        a_sb[:, ic * B : (ic + 1) * B],
                        start=(ic == 0),
                        stop=(ic == n_c - 1),
                    ), "pe")

            def ab_from(s_ps, tag):
                alpha = ap.tile([128, 64], f16, tag="al" + tag)
                beta = ap.tile([128, 64], f32, tag="be" + tag)
                tmpb = ap.tile([128, 32], f32, tag="tb" + tag)
                pin(nc.vector.tensor_scalar(
                    tmpb[:], s_ps[:, 32:64], k0, 0.0, ALU.mult, ALU.add
                ), "dve")
                for h in range(2):
                    hs = slice(h * 32, (h + 1) * 32)
                    pin(nc.vector.tensor_scalar(
                        alpha[:, hs], s_ps[:, 0:32], k2, 1.0, ALU.mult, ALU.add
                    ), "dve")
                    pin(nc.vector.scalar_tensor_tensor(
                        beta[:, hs], s_ps[:, 0:32], kb, tmpb[:], ALU.mult,
                        ALU.add
                    ), "dve")
                return alpha, beta

            def tail_half(z_ps, alpha, beta, bcol, a_t, off, h):
                """a_t[:, off:off+64] = relu(z+b)*alpha + k1*z + beta.
                Per-mg vj tiles so the two relus (ACT || DVE) don't
                serialize on a shared tile; combine split DVE/GpSimd."""
                vja = ap.tile([128, B], f16, tag=f"vja{h}")
                vjb = ap.tile([128, B], f16, tag=f"vjb{h}")
                t1 = ap.tile([128, 64], f32, tag=f"t1{h}")
                pin(nc.vector.tensor_scalar(
                    vjb[:], z_ps[:, B : 2 * B],
                    tmisc[:, bcol + 1 : bcol + 2], 0.0, ALU.add, ALU.max,
                ), "dve")
                pin(nc.scalar.activation(
                    vja[:], z_ps[:, 0:B], AF.Relu,
                    bias=tmisc[:, bcol : bcol + 1], scale=1.0,
                ), "act")
                pin(nc.vector.scalar_tensor_tensor(
                    t1[:], z_ps[:], k1, beta[:], ALU.mult, ALU.add
                ), "dve")
                pin(nc.vector.tensor_tensor(
                    vja[:], vja[:], alpha[:, 0:B], ALU.mult), "dve")
                pin(nc.vector.tensor_tensor(
                    a_t[:, off : off + B], vja[:], t1[:, 0:B], ALU.add), "dve")
                pin(nc.gpsimd.tensor_tensor(
                    vjb[:], vjb[:], alpha[:, B : 2 * B], ALU.mult), "gps")
                pin(nc.gpsimd.tensor_tensor(
                    a_t[:, off + B : off + 2 * B], vjb[:], t1[:, B : 2 * B],
                    ALU.add), "gps")

            def square(asq_view, a_view):
                pin(nc.gpsimd.tensor_tensor(
                    asq_view, a_view, a_view, ALU.mult
                ), "gps")

            # ---- SBUF activation tiles ----
            xsq = ap.tile([128, XW], f16, tag="xsq")
            a2 = ap.tile([128, 128], f16, tag="a2")
            asq2 = ap.tile([128, 128], f16, tag="asq2")
            a3 = ap.tile([128, 128], f16, tag="a3")
            asq3 = ap.tile([128, 128], f16, tag="asq3")

            # ================= layer 1 =================
            pin(nc.vector.tensor_tensor(xsq[:], tx, tx, ALU.mult), "dve")
            s1p = pp.tile([128, 64], f32, tag="s")
            z1a = pp.tile([128, 64], f32, tag="za")
            z1b = pp.tile([128, 64], f32, tag="zb")
            schain(s1p, 0, tx, 8)
            schain(s1p, B, xsq[:], 8)
            al1, be1 = ab_from(s1p, "1")
            zchain(z1a, 0, tw1[0][:], 0, tx, 8)
            zchain(z1a, B, tw1[0][:], 1024, tx, 8)
            tail_half(z1a, al1, be1, 0, a2, 0, 0)
            square(asq2[:, 0:64], a2[:, 0:64])
            zchain(z1b, 0, tw1[1][:], 0, tx, 8)
            zchain(z1b, B, tw1[1][:], 1024, tx, 8)
            tail_half(z1b, al1, be1, 2, a2, 64, 1)
            square(asq2[:, 64:128], a2[:, 64:128])

            # ================= layer 2 =================
            s2p = pp.tile([128, 64], f32, tag="s")
            z2a = pp.tile([128, 64], f32, tag="zc")
            z2b = pp.tile([128, 64], f32, tag="zd")
            schain(s2p, 0, a2[:], 4)
            schain(s2p, B, asq2[:], 4)
            al2, be2 = ab_from(s2p, "2")
            zchain(z2a, 0, tw2a[:], 0, a2[:], 4)
            zchain(z2a, B, tw2a[:], 512, a2[:], 4)
            tail_half(z2a, al2, be2, 4, a3, 0, 0)
            square(asq3[:, 0:64], a3[:, 0:64])
            zchain(z2b, 0, tw2b[:], 0, a2[:], 4)
            zchain(z2b, B, tw2b[:], 512, a2[:], 4)
            tail_half(z2b, al2, be2, 6, a3, 64, 1)
            square(asq3[:, 64:128], a3[:, 64:128])

            # ================= layer 3 =================
            s3p = pp.tile([128, 64], f32, tag="s")
            schain(s3p, 0, a3[:], 4)
            schain(s3p, B, asq3[:], 4)
            al3, be3 = ab_from(s3p, "3")
            z3 = pp.tile([O3L, B], f32, tag="z3")
            for ic in range(4):
                pin(nc.tensor.matmul(
                    z3[:],
                    tw3[:, ic * O3L : (ic + 1) * O3L],
                    a3[:, ic * B : (ic + 1) * B],
                    start=(ic == 0),
                    stop=(ic == 3),
                ), "pe")
            vj3 = ap.tile([O3L, B], f16, tag="vj3")
            t13 = ap.tile([O3L, B], f32, tag="t13")
            out_sb = ap.tile([O3L, B], f32, tag="o3")
            pin(nc.scalar.activation(
                vj3[:], z3[:], AF.Relu, bias=tmisc[0:O3L, 8:9], scale=1.0
            ), "act")
            pin(nc.vector.scalar_tensor_tensor(
                t13[:], z3[:], k1, be3[0:O3L, 0:B], ALU.mult, ALU.add
            ), "dve")
            pin(nc.vector.tensor_tensor(
                vj3[:], vj3[:], al3[0:O3L, 0:B], ALU.mult
            ), "dve")
            pin(nc.vector.tensor_tensor(out_sb[:], vj3[:], t13[:], ALU.add),
                "dve")

            nc.sync.dma_start(out_d[:], out_sb[:])

    nc.compile()
    return nc


def kernel(**inputs):
    from concourse.bass_utils import run_bass_kernel_spmd

    x = np.asarray(inputs["x"], dtype=np.float32)
    fc1_w = np.asarray(inputs["fc1_w"], dtype=np.float32)
    fc1_b = np.asarray(inputs["fc1_b"], dtype=np.float32)
    fc2_w = np.asarray(inputs["fc2_w"], dtype=np.float32)
    fc2_b = np.asarray(inputs["fc2_b"], dtype=np.float32)
    fc3_w = np.asarray(inputs["fc3_w"], dtype=np.float32)
    fc3_b = np.asarray(inputs["fc3_b"], dtype=np.float32)
    c1w = np.asarray(inputs["conv1_w"], dtype=np.float32)
    c1b = np.asarray(inputs["conv1_b"], dtype=np.float32)
    c2w = np.asarray(inputs["conv2_w"], dtype=np.float32)
    c2b = np.asarray(inputs["conv2_b"], dtype=np.float32)
    bn = float(np.asarray(inputs["batch_num"]).astype(np.float64))

    scale = np.float32(RATE) / np.float32(bn)
    g = (c1w.T @ c2w[0]).astype(np.float32)  # [3]
    hb = np.float32(c1b @ c2w[0] + c2b[0])
    k0 = float(scale * g[0])
    k1 = float(scale * g[1])
    k2 = float(scale * g[2])
    kb = float(scale * hb)

    key = (k0, k1, k2, kb)
    if key not in _CACHE:
        _CACHE[key] = _build(*key)
    nc = _CACHE[key]

    def pack_mg(Wt, n_c, mgs, mw):
        out = np.empty((128, len(mgs) * n_c * mw), dtype=np.float16)
        col = 0
        for mg in mgs:
            for ic in range(n_c):
                out[:, col : col + mw] = Wt[
                    ic * 128 : (ic + 1) * 128, mg * mw : (mg + 1) * mw
                ]
                col += mw
        return np.ascontiguousarray(out)

    xm_h = (
        x.T.reshape(8, 128, B).transpose(1, 0, 2).reshape(128, 8 * B)
    ).astype(np.float16)

    misc_h = np.zeros((128, 12), dtype=np.float32)
    misc_h[:, 0:4] = fc1_b.reshape(4, 128).T
    misc_h[:, 4:8] = fc2_b.reshape(4, 128).T

    w1t = fc1_w.T
    w1s_h = [pack_mg(w1t, 8, (2 * m, 2 * m + 1), 128) for m in range(2)]
    w2a_h = pack_mg(fc2_w.T, 4, (0, 1), 128)
    w2b_h = pack_mg(fc2_w.T, 4, (2, 3), 128)

    in_maps = []
    for c in range(N_CORES):
        w3_h = pack_mg(fc3_w[c * O3L : (c + 1) * O3L].T, 4, (0,), O3L)
        m_h = misc_h.copy()
        m_h[0:O3L, 8] = fc3_b[c * O3L : (c + 1) * O3L]
        lead_h = np.ascontiguousarray(
            np.concatenate([m_h.view(np.float16), xm_h], axis=1)
        )
        in_maps.append(
            dict(
                lead=lead_h, w1s0=w1s_h[0], w1s1=w1s_h[1],
                w2a=w2a_h, w2b=w2b_h, w3m=w3_h,
            )
        )

    res = run_bass_kernel_spmd(nc, in_maps, list(range(N_CORES)))
    global LAST_RESULTS
    LAST_RESULTS = res
    return np.ascontiguousarray(
        np.concatenate([res.results[c]["out"].T for c in range(N_CORES)], axis=1)
    ).astype(np.float32)


if __name__ == "__main__":
    rng = np.random.default_rng(0)

    def lin(fo, fi):
        bound = 1.0 / np.sqrt(fi)
        return (
            rng.uniform(-bound, bound, (fo, fi)).astype(np.float32),
            rng.uniform(-bound, bound, (fo,)).astype(np.float32),
        )

    fc1_w, fc1_b = lin(512, 1024)
    fc2_w, fc2_b = lin(512, 512)
    fc3_w, fc3_b = lin(256, 512)
    c1w, c1b = lin(8, 3)
    c2w, c2b = lin(1, 8)
    ins = dict(
        x=rng.standard_normal((32, 1024)).astype(np.float32),
        fc1_w=fc1_w, fc1_b=fc1_b, fc2_w=fc2_w, fc2_b=fc2_b,
        fc3_w=fc3_w, fc3_b=fc3_b,
        conv1_w=c1w, conv1_b=c1b, conv2_w=c2w, conv2_b=c2b,
        batch_num=10,
    )
    out = kernel(**ins)
    print("kernel out", out.shape, out.dtype, float(np.abs(out).max()))
